# revision 6
# baseline (speedup 1.0000x reference)
"""MoE top-1 routing kernel for Trainium2 — fp8 DoubleRow triple-split.

Per-core (expert-parallel) math, capacity C tokens:
  h = gelu((W1s.T Xs) * alpha + b1)         alpha = 1/(sx*sw1)
  y = (W2s.T Hs) / sw2                       (division folded into host combine)
with every matmul operand split hi(e4m3) + lo(e5m2) at a power-of-2 scale and
computed as three DoubleRow fp8 matmuls per 256-contraction group:
  hi@hi + hi@lo + lo@hi    (lo@lo ~ 0.06% -> dropped)
DoubleRow processes 256 contraction rows per pass at 0.5 cycles/row, so the
triple split runs at 4/3 the bf16/fp32r FLOP rate with ~3e-3 rel error.

Layouts (contraction k -> partition p, slot i, group t: k = t*256 + i*128 + p):
  x   [128, (ct, t, i, n)]   w1 [128, (jt, t, i, m128)]
  h   [128, (jt=jg*2+i, n256)]  w2 [128, (jg, i, d1024)]
y accumulates fully in PSUM per (dt, ct); no SBUF adds.
"""

import os
import sys

for _p in ("/opt/trn_rl_repo",):
    if _p not in sys.path:
        sys.path.append(_p)

import numpy as np
import ml_dtypes

D = 1024
H = 4096
E = 8
NP = 128
TG1 = D // 256   # 4  mm1 contraction groups
JT_N = H // 128  # 32 mm1 output j-tiles
JG_N = H // 256  # 16 mm2 contraction groups
DT_N = D // 128  # 8  mm2 output d-tiles

F8H = ml_dtypes.float8_e4m3   # TRN fp8e4 (max +-240)
F8L = ml_dtypes.float8_e5m2   # TRN fp8e5

_cache = {}


def _ctiles(C):
    """Column tiles: 256-wide pieces plus a mult-of-4 tail."""
    out = []
    c0 = 0
    while C - c0 >= 256:
        out.append((c0, 256))
        c0 += 256
    if C - c0:
        assert (C - c0) % 4 == 0
        out.append((c0, C - c0))
    return out


def _build(C, alpha, act=None):
    from contextlib import ExitStack

    import concourse.bass as bass  # noqa: F401
    import concourse.tile as tile
    from concourse import bacc, mybir

    f32 = mybir.dt.float32
    f32r = mybir.dt.float32r
    f8h = mybir.dt.float8e4
    f8l = mybir.dt.float8e5
    DR = mybir.MatmulPerfMode.DoubleRow
    GELU = (
        mybir.ActivationFunctionType.Gelu
        if act is None
        else getattr(mybir.ActivationFunctionType, act)
    )
    IDENT = mybir.ActivationFunctionType.Identity

    cts = _ctiles(C)
    # flat offset of (ct, t, i) block in the x layout
    xoff = []
    o = 0
    for (_, n) in cts:
        xoff.append(o)
        o += TG1 * 2 * n
    XW = o  # total x cols per partition

    nc = bacc.Bacc("TRN2", target_bir_lowering=False, debug=False, num_devices=E)
    xh_d = nc.dram_tensor("xh", [NP, XW], f8h, kind="ExternalInput").ap()
    xl_d = nc.dram_tensor("xl", [NP, XW], f8l, kind="ExternalInput").ap()
    w1h_d = nc.dram_tensor("w1h", [NP, JT_N * TG1 * 2 * 128], f8h,
                           kind="ExternalInput").ap()
    w1l_d = nc.dram_tensor("w1l", [NP, JT_N * TG1 * 2 * 128], f8l,
                           kind="ExternalInput").ap()
    w2h_d = nc.dram_tensor("w2h", [NP, JG_N * 2 * D], f8h,
                           kind="ExternalInput").ap()
    w2l_d = nc.dram_tensor("w2l", [NP, JG_N * 2 * D], f8l,
                           kind="ExternalInput").ap()
    b1_d = nc.dram_tensor("b1t", [NP, JT_N], f32, kind="ExternalInput").ap()
    y_d = nc.dram_tensor("yT", [D, C], f32, kind="ExternalOutput").ap()

    with tile.TileContext(nc) as tc, ExitStack() as ctx:
        xp = ctx.enter_context(tc.tile_pool(name="x", bufs=1))
        w1p = ctx.enter_context(tc.tile_pool(name="w1", bufs=1))
        w2p = ctx.enter_context(tc.tile_pool(name="w2", bufs=1))
        hp = ctx.enter_context(tc.tile_pool(name="h", bufs=2))
        hsp = ctx.enter_context(tc.tile_pool(name="hs", bufs=6))
        ytp = ctx.enter_context(tc.tile_pool(name="yt", bufs=6))
        bp = ctx.enter_context(tc.tile_pool(name="b", bufs=1))
        psp = ctx.enter_context(tc.tile_pool(name="psp", bufs=8, space="PSUM"))

        xh = xp.tile([NP, XW], f8h)
        xl = xp.tile([NP, XW], f8l)
        w1h = w1p.tile([NP, JT_N * TG1 * 2 * 128], f8h)
        w1l = w1p.tile([NP, JT_N * TG1 * 2 * 128], f8l)
        w2h = w2p.tile([NP, JG_N * 2 * D], f8h)
        w2l = w2p.tile([NP, JG_N * 2 * D], f8l)
        b1t = bp.tile([NP, JT_N], f32)

        # PE warmup: chew zero matmuls so the clock ramps while DMAs land.
        n_warm = int(os.environ.get("KWARM", "0"))
        if n_warm:
            warm = bp.tile([NP, 256], f32r, tag="warm")
            nc.gpsimd.memzero(warm[:])
            for _ in range(n_warm):
                wps = psp.tile([NP, 512], f32, tag="ps")
                nc.tensor.matmul(wps[:, :256], warm[:, :NP], warm[:],
                                 start=True, stop=True)

        # DMA issue. scalar queue: b1 + x (ct0 first).  sync queue: all of
        # w1 then all of w2 — w1 must stream ahead of mm1's consumption, w2
        # is only needed from mm2(ct0) (~40us in).
        # DMA plan. HWDGE costs a fixed 625ns per DMA instruction on a
        # globally-serialized device, so weights go in a few big chunks on
        # the SP queue; x/b1/y ride the separate SWDGE path (gpsimd/Pool).
        def dma_x(ci):
            o, nn8 = xoff[ci], TG1 * 2 * cts[ci][1]
            nc.gpsimd.dma_start(xh[:, o:o + nn8], xh_d[:, o:o + nn8])
            nc.gpsimd.dma_start(xl[:, o:o + nn8], xl_d[:, o:o + nn8])

        def dma_x_sc(ci):
            o, nn8 = xoff[ci], TG1 * 2 * cts[ci][1]
            nc.scalar.dma_start(xh[:, o:o + nn8], xh_d[:, o:o + nn8])
            nc.scalar.dma_start(xl[:, o:o + nn8], xl_d[:, o:o + nn8])

        def dma_w1(j0, w):
            # first few lo chunks ride the scalar queue: parallel SEQ issue
            # halves the early per-jt serialization (only HWDGE is shared);
            # later chunks stay on SP so gelu issue isn't pushed back
            s = slice(j0 * TG1 * 256, (j0 + w) * TG1 * 256)
            nc.sync.dma_start(w1h[:, s], w1h_d[:, s])
            eng = nc.scalar if j0 < 8 else nc.sync
            eng.dma_start(w1l[:, s], w1l_d[:, s])

        def dma_w2(g0, w):
            s = slice(g0 * 2 * D, (g0 + w) * 2 * D)
            nc.sync.dma_start(w2h[:, s], w2h_d[:, s])
            nc.sync.dma_start(w2l[:, s], w2l_d[:, s])

        # x(ct0)+b1 ride the SWDGE path in parallel with w1c0 on SP/HWDGE so
        # the first matmul group is gated by ~5us of stream, not 8. w1 chunks
        # are paced ahead of the jt-interleaved first phase's consumption
        # (~1.28us/jt); w2 rides the w1 tail so mm2(0)'s jg-ordered
        # consumption never catches the stream.
        dma_x(0)
        nc.gpsimd.dma_start(b1t[:], b1_d[:])
        if len(cts) > 1:
            dma_x_sc(1)
        for (j0, w) in [(0, 2), (2, 2), (4, 2), (6, 2), (8, 4), (12, 4),
                        (16, 4), (20, 4)]:
            dma_w1(j0, w)
        dma_w2(0, 4)
        dma_w1(24, 4)
        dma_w2(4, 4)
        dma_w1(28, 4)
        dma_w2(8, 4)
        dma_w2(12, 4)

        def x_ap(xt, ci, t):
            o = xoff[ci] + t * 2 * cts[ci][1]
            n = cts[ci][1]
            return xt[:, o:o + 2 * n].rearrange("p (i c) -> p i c", i=2)

        def w1_ap(wt, jt, t):
            o = jt * TG1 * 256 + t * 256
            return wt[:, o:o + 256].rearrange("p (i m) -> p i m", i=2)

        def w2_ap(wt, jg, dt):
            o = jg * 2 * D
            return wt[:, o:o + 2 * D].rearrange("p (i d) -> p i d", i=2)[
                :, :, dt * 128:(dt + 1) * 128]

        def h_ap(ht, jg, n):
            o = jg * 2 * 256
            return ht[:, o:o + 512].rearrange("p (i c) -> p i c", i=2)[:, :, :n]

        hbufs = {}

        def alloc_h(ci):
            hht = hp.tile([NP, JT_N * 256], f8h, tag="hh", name=f"hh{ci}")
            hlt = hp.tile([NP, JT_N * 256], f8l, tag="hl", name=f"hl{ci}")
            hbufs[ci] = (hht, hlt)

        def mm1_jt(ci, jt):
            c0, n = cts[ci]
            hht, hlt = hbufs[ci]
            ps = psp.tile([NP, 512], f32, tag="ps", name=f"ps{ci}_{jt}")
            for t in range(TG1):
                nc.tensor.matmul(ps[:, :n], w1_ap(w1h, jt, t), x_ap(xh, ci, t),
                                 start=(t == 0), stop=False, perf_mode=DR)
                nc.tensor.matmul(ps[:, :n], w1_ap(w1h, jt, t), x_ap(xl, ci, t),
                                 start=False, stop=False, perf_mode=DR)
                nc.tensor.matmul(ps[:, :n], w1_ap(w1l, jt, t), x_ap(xh, ci, t),
                                 start=False, stop=(t == TG1 - 1), perf_mode=DR)
            h32 = hsp.tile([NP, 256], f32, tag="h32", name=f"h32_{ci}_{jt}")
            nc.scalar.activation(h32[:, :n], ps[:, :n], GELU,
                                 bias=b1t[:, jt:jt + 1], scale=alpha)
            hs = slice(jt * 256, jt * 256 + n)
            nc.gpsimd.tensor_copy(hht[:, hs], h32[:, :n])
            nc.vector.tensor_sub(hlt[:, hs], h32[:, :n], hht[:, hs])

        def mm1(ci):
            alloc_h(ci)
            for jt in range(JT_N):
                mm1_jt(ci, jt)

        def mm1_pair(ci0, ci1, late_x=False):
            # jt-interleaved: w1[jt] is consumed at the paced ~1.28us/jt the
            # weight stream can sustain, keeping the PE fed from the start.
            alloc_h(ci0)
            alloc_h(ci1)
            for jt in range(JT_N):
                mm1_jt(ci0, jt)
                mm1_jt(ci1, jt)
                if late_x and jt == 15 and len(cts) > 2:
                    # late x for the remaining c-tiles: two big transfers on
                    # the scalar queue, issued mid-phase behind the acts
                    o = xoff[2]
                    nc.scalar.dma_start(xh[:, o:XW], xh_d[:, o:XW])
                    nc.scalar.dma_start(xl[:, o:XW], xl_d[:, o:XW])

        def emit_y(ci, dt, ps):
            c0, n = cts[ci]
            yt = ytp.tile([NP, 256], f32, tag="yt")
            nc.vector.tensor_copy(yt[:, :n], ps[:, :n])
            # HWDGE queues are idle once the weights have streamed (the SWDGE
            # generator at ~1us/DMA would serialize the final drain);
            # alternate SP/scalar so the per-queue SEQ time overlaps too.
            eng = nc.sync if dt % 2 == 0 else nc.scalar
            eng.dma_start(y_d[dt * 128:(dt + 1) * 128, c0:c0 + n],
                          yt[:, :n])

        def mm2_jg(ci):
            # jg-outer with all 8 d-tile accumulators open: w2 is consumed
            # incrementally so the streaming weights never stall the PE.
            c0, n = cts[ci]
            hht, hlt = hbufs.pop(ci)
            ys = [psp.tile([NP, 512], f32, tag="ps", name=f"y{ci}_{dt}")
                  for dt in range(DT_N)]
            for jg in range(JG_N):
                for dt in range(DT_N):
                    ps = ys[dt]
                    nc.tensor.matmul(ps[:, :n], w2_ap(w2h, jg, dt), h_ap(hht, jg, n),
                                     start=(jg == 0), stop=False, perf_mode=DR)
                    nc.tensor.matmul(ps[:, :n], w2_ap(w2h, jg, dt), h_ap(hlt, jg, n),
                                     start=False, stop=False, perf_mode=DR)
                    nc.tensor.matmul(ps[:, :n], w2_ap(w2l, jg, dt), h_ap(hht, jg, n),
                                     start=False, stop=(jg == JG_N - 1), perf_mode=DR)
            for dt in range(DT_N):
                emit_y(ci, dt, ys[dt])

        def mm2(ci):
            # dt-outer: frees PSUM banks incrementally so the next mm1 phase
            # never waits on them (weights fully resident by now).
            c0, n = cts[ci]
            hht, hlt = hbufs.pop(ci)
            for dt in range(DT_N):
                ps = psp.tile([NP, 512], f32, tag="ps", name=f"y{ci}_{dt}")
                for jg in range(JG_N):
                    nc.tensor.matmul(ps[:, :n], w2_ap(w2h, jg, dt), h_ap(hht, jg, n),
                                     start=(jg == 0), stop=False, perf_mode=DR)
                    nc.tensor.matmul(ps[:, :n], w2_ap(w2h, jg, dt), h_ap(hlt, jg, n),
                                     start=False, stop=False, perf_mode=DR)
                    nc.tensor.matmul(ps[:, :n], w2_ap(w2l, jg, dt), h_ap(hht, jg, n),
                                     start=False, stop=(jg == JG_N - 1), perf_mode=DR)
                emit_y(ci, dt, ps)

        # schedule: pair the first two c-tiles so mm1 work covers the weight
        # stream, then steady mm1/mm2 alternation.
        NCT = len(cts)
        # pair the small tail tile with its predecessor: at n<256 the act/DVE
        # per-tile overheads outrun the PE unless interleaved with full tiles
        tail_pair = NCT >= 4 and cts[-1][1] < 256
        if NCT >= 2:
            mm1_pair(0, 1, late_x=True)
            mm2_jg(0)
            mm2(1)
            singles = list(range(2, NCT - 2 if tail_pair else NCT))
        else:
            singles = list(range(NCT))
        for ci in singles:
            mm1(ci)
            mm2(ci)
        if tail_pair:
            mm1_pair(NCT - 2, NCT - 1)
            mm2(NCT - 2)
            mm2(NCT - 1)

    nc.compile()
    return nc


def _get_nc(C, alpha, act=None):
    key = (C, float(alpha), act)
    if key not in _cache:
        _cache[key] = _build(C, alpha, act)
    return _cache[key]


def _pow2_scale(absmax):
    return float(2.0 ** np.floor(np.log2(224.0 / absmax)))


def _split_fp8(a):
    hi = np.clip(a, -240, 240).astype(F8H)
    lo = (a - hi.astype(np.float32)).astype(F8L)
    return hi, lo


def _lay_x(Xq, cts):
    """[D, C] -> [128, (ct, t, i, n)]"""
    a = Xq.reshape(TG1, 2, NP, Xq.shape[1]).transpose(2, 0, 1, 3)  # p t i c
    return np.concatenate(
        [a[:, :, :, c0:c0 + n].reshape(NP, -1) for (c0, n) in cts], axis=1
    )


def _lay_w1(Wq):
    """[D, H] -> [128, (jt, t, i, m)]"""
    return np.ascontiguousarray(
        Wq.reshape(TG1, 2, NP, JT_N, 128).transpose(2, 3, 0, 1, 4).reshape(NP, -1)
    )


def _lay_w2(Wq):
    """[H, D] -> [128, (jg, i, d)]"""
    return np.ascontiguousarray(
        Wq.reshape(JG_N, 2, NP, D).transpose(2, 0, 1, 3).reshape(NP, -1)
    )


def _route(xf, gate_w, gate_b):
    logits = xf @ gate_w + gate_b
    m = logits.max(-1, keepdims=True)
    ex = np.exp(logits - m)
    pb = ex / ex.sum(-1, keepdims=True)
    idx = logits.argmax(-1)
    wgt = pb[np.arange(pb.shape[0]), idx]
    return idx, wgt


_jit_cache = {}


def _run(nc, in_maps):
    """Execute nc on the 8 cores via PJRT, caching the jitted executable."""
    import jax
    from jax.sharding import Mesh, PartitionSpec
    from jax.experimental.shard_map import shard_map
    from concourse import bass2jax, mybir

    key = id(nc)
    if key not in _jit_cache:
        bass2jax.install_neuronx_cc_hook()
        pid_name = nc.partition_id_tensor.name if nc.partition_id_tensor else None
        in_names, out_names, out_avals = [], [], []
        for alloc in nc.m.functions[0].allocations:
            if not isinstance(alloc, mybir.MemoryLocationSet):
                continue
            name = alloc.memorylocations[0].name
            if alloc.kind == "ExternalInput":
                if name != pid_name:
                    in_names.append(name)
            elif alloc.kind == "ExternalOutput":
                out_names.append(name)
                out_avals.append(
                    jax.core.ShapedArray(
                        tuple(alloc.tensor_shape), mybir.dt.np(alloc.dtype)
                    )
                )
        n_params = len(in_names)
        all_names = in_names + out_names
        if pid_name is not None:
            all_names = all_names + [pid_name]

        def _body(*args):
            operands = list(args)
            if pid_name is not None:
                operands.append(bass2jax.partition_id_tensor())
            return tuple(
                bass2jax._bass_exec_p.bind(
                    *operands,
                    out_avals=tuple(out_avals),
                    in_names=tuple(all_names),
                    out_names=tuple(out_names),
                    lowering_input_output_aliases=(),
                    sim_require_finite=True,
                    sim_require_nnan=True,
                    nc=nc,
                )
            )

        mesh = Mesh(np.asarray(jax.devices()[:E]), ("core",))
        nio = n_params + len(out_names)
        sharded = jax.jit(
            shard_map(
                _body,
                mesh=mesh,
                in_specs=(PartitionSpec("core"),) * nio,
                out_specs=(PartitionSpec("core"),) * len(out_names),
                check_rep=False,
            ),
            donate_argnums=tuple(range(n_params, nio)),
            keep_unused=True,
        )
        _jit_cache[key] = (sharded, in_names, out_names, out_avals)

    sharded, in_names, out_names, out_avals = _jit_cache[key]
    concat_in = [
        np.concatenate([np.asarray(m[name]) for m in in_maps], axis=0)
        for name in in_names
    ]
    concat_zeros = [
        np.zeros((E * av.shape[0], *av.shape[1:]), av.dtype) for av in out_avals
    ]
    outs = sharded(*concat_in, *concat_zeros)
    return [
        {
            name: np.asarray(outs[i]).reshape(E, *out_avals[i].shape)[c]
            for i, name in enumerate(out_names)
        }
        for c in range(E)
    ]


def kernel(x, gate_w, gate_b, w1, b1, w2, b2):
    x = np.asarray(x, np.float32)
    gate_w = np.asarray(gate_w, np.float32)
    gate_b = np.asarray(gate_b, np.float32)
    w1 = np.asarray(w1, np.float32)
    b1 = np.asarray(b1, np.float32)
    w2 = np.asarray(w2, np.float32)
    b2 = np.asarray(b2, np.float32)

    b, s, d = x.shape
    T = b * s
    xf = x.reshape(T, d)

    idx, wgt = _route(xf, gate_w, gate_b)

    sx = _pow2_scale(np.abs(xf).max())
    sw1 = _pow2_scale(np.abs(w1).max())
    sw2 = _pow2_scale(np.abs(w2).max())
    alpha = 1.0 / (sx * sw1)

    tids_all = [np.nonzero(idx == e)[0] for e in range(E)]
    maxc = max(len(t) for t in tids_all)
    CMAX = 1280
    n_chunks = max(1, -(-maxc // CMAX))

    out = np.empty((T, D), np.float32)
    for ci in range(n_chunks):
        tids = [t[ci * CMAX:(ci + 1) * CMAX] for t in tids_all]
        mc = max(len(t) for t in tids)
        C = max(256, -(-mc // 4) * 4)
        cts = _ctiles(C)

        nc = _get_nc(C, alpha)

        in_maps = []
        for e in range(E):
            xT = np.zeros((D, C), np.float32)
            n = len(tids[e])
            xT[:, :n] = xf[tids[e]].T * sx
            xqh, xql = _split_fp8(xT)
            w1h, w1l = _split_fp8(w1[e] * sw1)
            w2h, w2l = _split_fp8(w2[e] * sw2)
            in_maps.append({
                "xh": _lay_x(xqh, cts),
                "xl": _lay_x(xql, cts),
                "w1h": _lay_w1(w1h),
                "w1l": _lay_w1(w1l),
                "w2h": _lay_w2(w2h),
                "w2l": _lay_w2(w2l),
                "b1t": np.ascontiguousarray(b1[e].reshape(JT_N, NP).T),
            })

        res = _run(nc, in_maps)

        for e in range(E):
            n = len(tids[e])
            if n:
                y = res[e]["yT"][:, :n].T / sw2  # [n, D]
                out[tids[e]] = wgt[tids[e], None] * (y + b2[e])
    return out.reshape(b, s, d)


# revision 8
# speedup vs baseline: 1.1501x; 1.1501x over previous
"""MoE top-1 routing kernel for Trainium2 — fp8 DoubleRow triple-split.

Per-core (expert-parallel) math, capacity C tokens:
  h = gelu((W1s.T Xs) * alpha + b1)         alpha = 1/(sx*sw1)
  y = (W2s.T Hs) / sw2                       (division folded into host combine)
with every matmul operand split hi(e4m3) + lo(e5m2) at a power-of-2 scale and
computed as three DoubleRow fp8 matmuls per 256-contraction group:
  hi@hi + hi@lo + lo@hi    (lo@lo ~ 0.06% -> dropped)
DoubleRow processes 256 contraction rows per pass at 0.5 cycles/row, so the
triple split runs at 4/3 the bf16/fp32r FLOP rate with ~3e-3 rel error.

Layouts (contraction k -> partition p, slot i, group t: k = t*256 + i*128 + p):
  x   [128, (ct, t, i, n)]   w1 [128, (jt, t, i, m128)]
  h   [128, (jt=jg*2+i, n256)]  w2 [128, (jg, i, d1024)]
y accumulates fully in PSUM per (dt, ct); no SBUF adds.
"""

import os
import sys

for _p in ("/opt/trn_rl_repo",):
    if _p not in sys.path:
        sys.path.append(_p)

import numpy as np
import ml_dtypes

D = 1024
H = 4096
E = 8
NP = 128
TG1 = D // 256   # 4  mm1 contraction groups
JT_N = H // 128  # 32 mm1 output j-tiles
JG_N = H // 256  # 16 mm2 contraction groups
DT_N = D // 128  # 8  mm2 output d-tiles

F8H = ml_dtypes.float8_e4m3   # TRN fp8e4 (max +-240)
F8L = ml_dtypes.float8_e5m2   # TRN fp8e5

_cache = {}


def _ctiles(C):
    """Column tiles: 256-wide pieces plus a mult-of-4 tail."""
    out = []
    c0 = 0
    while C - c0 >= 256:
        out.append((c0, 256))
        c0 += 256
    if C - c0:
        assert (C - c0) % 4 == 0
        out.append((c0, C - c0))
    return out


def _build(P, Q, alpha, act=None):
    from contextlib import ExitStack

    import concourse.bass as bass  # noqa: F401
    import concourse.tile as tile
    from concourse import bacc, mybir

    f32 = mybir.dt.float32
    f32r = mybir.dt.float32r
    f8h = mybir.dt.float8e4
    f8l = mybir.dt.float8e5
    DR = mybir.MatmulPerfMode.DoubleRow
    GELU = (
        mybir.ActivationFunctionType.Gelu
        if act is None
        else getattr(mybir.ActivationFunctionType, act)
    )
    IDENT = mybir.ActivationFunctionType.Identity

    # precise tiles (3-term) then cheap tiles (2-term, w-lo dropped)
    cts = [(c0, n, False) for (c0, n) in _ctiles(P)] + [
        (P + c0, n, True) for (c0, n) in _ctiles(Q)]
    C = P + Q
    # flat offset of (ct, t, i) block in the x layout
    xoff = []
    o = 0
    for (_, n, _ch) in cts:
        xoff.append(o)
        o += TG1 * 2 * n
    XW = o  # total x cols per partition

    nc = bacc.Bacc("TRN2", target_bir_lowering=False, debug=False, num_devices=E)
    xh_d = nc.dram_tensor("xh", [NP, XW], f8h, kind="ExternalInput").ap()
    xl_d = nc.dram_tensor("xl", [NP, XW], f8l, kind="ExternalInput").ap()
    w1h_d = nc.dram_tensor("w1h", [NP, JT_N * TG1 * 2 * 128], f8h,
                           kind="ExternalInput").ap()
    w1l_d = nc.dram_tensor("w1l", [NP, JT_N * TG1 * 2 * 128], f8l,
                           kind="ExternalInput").ap()
    w2h_d = nc.dram_tensor("w2h", [NP, JG_N * 2 * D], f8h,
                           kind="ExternalInput").ap()
    w2l_d = nc.dram_tensor("w2l", [NP, JG_N * 2 * D], f8l,
                           kind="ExternalInput").ap()
    b1_d = nc.dram_tensor("b1t", [NP, JT_N], f32, kind="ExternalInput").ap()
    y_d = nc.dram_tensor("yT", [D, C], f32, kind="ExternalOutput").ap()

    with tile.TileContext(nc) as tc, ExitStack() as ctx:
        xp = ctx.enter_context(tc.tile_pool(name="x", bufs=1))
        w1p = ctx.enter_context(tc.tile_pool(name="w1", bufs=1))
        w2p = ctx.enter_context(tc.tile_pool(name="w2", bufs=1))
        hp = ctx.enter_context(tc.tile_pool(name="h", bufs=2))
        hsp = ctx.enter_context(tc.tile_pool(name="hs", bufs=int(__import__("os").environ.get("HSB", "6"))))
        ytp = ctx.enter_context(tc.tile_pool(name="yt", bufs=int(__import__("os").environ.get("YTB", "6"))))
        bp = ctx.enter_context(tc.tile_pool(name="b", bufs=1))
        psp = ctx.enter_context(tc.tile_pool(name="psp", bufs=8, space="PSUM"))

        xh = xp.tile([NP, XW], f8h)
        xl = xp.tile([NP, XW], f8l)
        w1h = w1p.tile([NP, JT_N * TG1 * 2 * 128], f8h)
        w1l = w1p.tile([NP, JT_N * TG1 * 2 * 128], f8l)
        w2h = w2p.tile([NP, JG_N * 2 * D], f8h)
        w2l = w2p.tile([NP, JG_N * 2 * D], f8l)
        b1t = bp.tile([NP, JT_N], f32)

        # PE warmup: chew zero matmuls so the clock ramps while DMAs land.
        n_warm = int(os.environ.get("KWARM", "0"))
        if n_warm:
            warm = bp.tile([NP, 256], f32r, tag="warm")
            nc.gpsimd.memzero(warm[:])
            for _ in range(n_warm):
                wps = psp.tile([NP, 512], f32, tag="ps")
                nc.tensor.matmul(wps[:, :256], warm[:, :NP], warm[:],
                                 start=True, stop=True)

        # DMA issue. scalar queue: b1 + x (ct0 first).  sync queue: all of
        # w1 then all of w2 — w1 must stream ahead of mm1's consumption, w2
        # is only needed from mm2(ct0) (~40us in).
        # DMA plan. HWDGE costs a fixed 625ns per DMA instruction on a
        # globally-serialized device, so weights go in a few big chunks on
        # the SP queue; x/b1/y ride the separate SWDGE path (gpsimd/Pool).
        def dma_x(ci):
            o, nn8 = xoff[ci], TG1 * 2 * cts[ci][1]
            nc.gpsimd.dma_start(xh[:, o:o + nn8], xh_d[:, o:o + nn8])
            nc.gpsimd.dma_start(xl[:, o:o + nn8], xl_d[:, o:o + nn8])

        def dma_x_sc(ci):
            o, nn8 = xoff[ci], TG1 * 2 * cts[ci][1]
            nc.scalar.dma_start(xh[:, o:o + nn8], xh_d[:, o:o + nn8])
            nc.scalar.dma_start(xl[:, o:o + nn8], xl_d[:, o:o + nn8])

        def dma_w1(j0, w):
            # first few lo chunks ride the scalar queue: parallel SEQ issue
            # halves the early per-jt serialization (only HWDGE is shared);
            # later chunks stay on SP so gelu issue isn't pushed back
            s = slice(j0 * TG1 * 256, (j0 + w) * TG1 * 256)
            nc.sync.dma_start(w1h[:, s], w1h_d[:, s])
            eng = nc.scalar if j0 < int(os.environ.get("W1LSC", "8")) else nc.sync
            eng.dma_start(w1l[:, s], w1l_d[:, s])

        def dma_w2(g0, w):
            s = slice(g0 * 2 * D, (g0 + w) * 2 * D)
            nc.sync.dma_start(w2h[:, s], w2h_d[:, s])
            nc.sync.dma_start(w2l[:, s], w2l_d[:, s])

        # x(ct0)+b1 ride the SWDGE path in parallel with w1c0 on SP/HWDGE so
        # the first matmul group is gated by ~5us of stream, not 8. w1 chunks
        # are paced ahead of the jt-interleaved first phase's consumption
        # (~1.28us/jt); w2 rides the w1 tail so mm2(0)'s jg-ordered
        # consumption never catches the stream.
        _xq = os.environ.get("XQ", "0")
        if _xq == "0":
            dma_x(0)
            nc.gpsimd.dma_start(b1t[:], b1_d[:])
            if len(cts) > 1:
                dma_x_sc(1)
        elif _xq == "1":
            # x0: h on swdge, l on SP ahead of w1; x1 split swdge/scalar
            o, nn8 = xoff[0], TG1 * 2 * cts[0][1]
            nc.gpsimd.dma_start(xh[:, o:o + nn8], xh_d[:, o:o + nn8])
            nc.sync.dma_start(xl[:, o:o + nn8], xl_d[:, o:o + nn8])
            nc.gpsimd.dma_start(b1t[:], b1_d[:])
            if len(cts) > 1:
                o, nn8 = xoff[1], TG1 * 2 * cts[1][1]
                nc.gpsimd.dma_start(xh[:, o:o + nn8], xh_d[:, o:o + nn8])
                nc.scalar.dma_start(xl[:, o:o + nn8], xl_d[:, o:o + nn8])
        else:
            # x0 h+l both on SP first, x1 on scalar, b1 swdge
            o, nn8 = xoff[0], TG1 * 2 * cts[0][1]
            nc.sync.dma_start(xh[:, o:o + nn8], xh_d[:, o:o + nn8])
            nc.sync.dma_start(xl[:, o:o + nn8], xl_d[:, o:o + nn8])
            nc.gpsimd.dma_start(b1t[:], b1_d[:])
            if len(cts) > 1:
                dma_x_sc(1)
        _w1ch = [int(v) for v in os.environ.get("W1CH", "2,2,2,2,4,4,4,4").split(",")]
        j0 = 0
        for w in _w1ch:
            dma_w1(j0, w)
            j0 += w
        assert j0 == 24
        dma_w2(0, 4)
        dma_w1(24, 4)
        dma_w2(4, 4)
        dma_w1(28, 4)
        dma_w2(8, 4)
        dma_w2(12, 4)

        def x_ap(xt, ci, t):
            o = xoff[ci] + t * 2 * cts[ci][1]
            n = cts[ci][1]
            return xt[:, o:o + 2 * n].rearrange("p (i c) -> p i c", i=2)

        def w1_ap(wt, jt, t):
            o = jt * TG1 * 256 + t * 256
            return wt[:, o:o + 256].rearrange("p (i m) -> p i m", i=2)

        def w2_ap(wt, jg, dt):
            o = jg * 2 * D
            return wt[:, o:o + 2 * D].rearrange("p (i d) -> p i d", i=2)[
                :, :, dt * 128:(dt + 1) * 128]

        def h_ap(ht, jg, n):
            o = jg * 2 * 256
            return ht[:, o:o + 512].rearrange("p (i c) -> p i c", i=2)[:, :, :n]

        hbufs = {}

        def alloc_h(ci):
            hht = hp.tile([NP, JT_N * 256], f8h, tag="hh", name=f"hh{ci}")
            hlt = hp.tile([NP, JT_N * 256], f8l, tag="hl", name=f"hl{ci}")
            hbufs[ci] = (hht, hlt)

        def mm1_jt(ci, jt):
            c0, n, ch = cts[ci]
            hht, hlt = hbufs[ci]
            ps = psp.tile([NP, 512], f32, tag="ps", name=f"ps{ci}_{jt}")
            for t in range(TG1):
                terms = [(w1h, xh), (w1h, xl)] + ([] if ch else [(w1l, xh)])
                for k, (wt, xt) in enumerate(terms):
                    nc.tensor.matmul(ps[:, :n], w1_ap(wt, jt, t), x_ap(xt, ci, t),
                                     start=(t == 0 and k == 0),
                                     stop=(t == TG1 - 1 and k == len(terms) - 1),
                                     perf_mode=DR)
            h32 = hsp.tile([NP, 256], f32, tag="h32", name=f"h32_{ci}_{jt}")  # noqa
            nc.scalar.activation(h32[:, :n], ps[:, :n], GELU,
                                 bias=b1t[:, jt:jt + 1], scale=alpha)
            hs = slice(jt * 256, jt * 256 + n)
            nc.gpsimd.tensor_copy(hht[:, hs], h32[:, :n])
            nc.vector.tensor_sub(hlt[:, hs], h32[:, :n], hht[:, hs])

        def mm1(ci):
            alloc_h(ci)
            for jt in range(JT_N):
                mm1_jt(ci, jt)

        def mm1_pair(ci0, ci1, late_x=False):
            # jt-interleaved: w1[jt] is consumed at the paced ~1.28us/jt the
            # weight stream can sustain, keeping the PE fed from the start.
            alloc_h(ci0)
            alloc_h(ci1)
            for jt in range(JT_N):
                mm1_jt(ci0, jt)
                mm1_jt(ci1, jt)
                if late_x and jt == 15 and len(cts) > 2:
                    # late x for the remaining c-tiles: two big transfers on
                    # the scalar queue, issued mid-phase behind the acts
                    o = xoff[2]
                    nc.scalar.dma_start(xh[:, o:XW], xh_d[:, o:XW])
                    nc.scalar.dma_start(xl[:, o:XW], xl_d[:, o:XW])

        def emit_y(ci, dt, ps):
            c0, n = cts[ci][0], cts[ci][1]
            yt = ytp.tile([NP, 256], f32, tag="yt")
            nc.vector.tensor_copy(yt[:, :n], ps[:, :n])
            # HWDGE queues are idle once the weights have streamed (the SWDGE
            # generator at ~1us/DMA would serialize the final drain);
            # alternate SP/scalar so the per-queue SEQ time overlaps too.
            eng = nc.sync if dt % 2 == 0 else nc.scalar
            eng.dma_start(y_d[dt * 128:(dt + 1) * 128, c0:c0 + n],
                          yt[:, :n])

        def mm2_terms(ci):
            hht, hlt = hbufs[ci]
            ch = cts[ci][2]
            return [(w2h, hht), (w2h, hlt)] + ([] if ch else [(w2l, hht)])

        def mm2_jg(ci):
            # jg-outer with all 8 d-tile accumulators open: w2 is consumed
            # incrementally so the streaming weights never stall the PE.
            c0, n, _ch = cts[ci]
            terms = mm2_terms(ci)
            hbufs.pop(ci)
            ys = [psp.tile([NP, 512], f32, tag="ps", name=f"y{ci}_{dt}")
                  for dt in range(DT_N)]
            for jg in range(JG_N):
                for dt in range(DT_N):
                    for k, (wt, ht) in enumerate(terms):
                        nc.tensor.matmul(ys[dt][:, :n], w2_ap(wt, jg, dt),
                                         h_ap(ht, jg, n),
                                         start=(jg == 0 and k == 0),
                                         stop=(jg == JG_N - 1 and k == len(terms) - 1),
                                         perf_mode=DR)
            for dt in range(DT_N):
                emit_y(ci, dt, ys[dt])

        def mm2(ci):
            # dt-outer: frees PSUM banks incrementally so the next mm1 phase
            # never waits on them (weights fully resident by now).
            c0, n, _ch = cts[ci]
            terms = mm2_terms(ci)
            hbufs.pop(ci)
            for dt in range(DT_N):
                ps = psp.tile([NP, 512], f32, tag="ps", name=f"y{ci}_{dt}")
                for jg in range(JG_N):
                    for k, (wt, ht) in enumerate(terms):
                        nc.tensor.matmul(ps[:, :n], w2_ap(wt, jg, dt),
                                         h_ap(ht, jg, n),
                                         start=(jg == 0 and k == 0),
                                         stop=(jg == JG_N - 1 and k == len(terms) - 1),
                                         perf_mode=DR)
                emit_y(ci, dt, ps)

        # schedule: pair the first two c-tiles so mm1 work covers the weight
        # stream, then steady mm1/mm2 alternation.
        def mm1_tri(ci0, ci1, ci2):
            alloc_h(ci0)
            alloc_h(ci1)
            alloc_h(ci2)
            for jt in range(JT_N):
                mm1_jt(ci0, jt)
                mm1_jt(ci1, jt)
                mm1_jt(ci2, jt)
                if jt == 15 and len(cts) > 3:
                    o = xoff[3]
                    nc.scalar.dma_start(xh[:, o:XW], xh_d[:, o:XW])
                    nc.scalar.dma_start(xl[:, o:XW], xl_d[:, o:XW])

        NCT = len(cts)
        # pairs keep act/DVE interleaved with PE work (cheap tiles and small
        # tails are activation-bound when run alone)
        if NCT >= 2:
            mm1_pair(0, 1, late_x=True)
            mm2_jg(0)
            mm2(1)
            i = 2
            while i + 1 < NCT:
                mm1_pair(i, i + 1)
                mm2(i)
                mm2(i + 1)
                i += 2
            if i < NCT:
                mm1(i)
                mm2(i)
        elif NCT == 1:
            mm1(0)
            mm2(0)

    nc.compile()
    return nc


def _get_nc(P, Q, alpha, act=None):
    key = (P, Q, float(alpha), act)
    if key not in _cache:
        _cache[key] = _build(P, Q, alpha, act)
    return _cache[key]


def _pow2_scale(absmax):
    return float(2.0 ** np.floor(np.log2(224.0 / absmax)))


def _split_fp8(a):
    hi = np.clip(a, -240, 240).astype(F8H)
    lo = (a - hi.astype(np.float32)).astype(F8L)
    return hi, lo


def _lay_x(Xq, cts):
    """[D, C] -> [128, (ct, t, i, n)]"""
    a = Xq.reshape(TG1, 2, NP, Xq.shape[1]).transpose(2, 0, 1, 3)  # p t i c
    return np.concatenate(
        [a[:, :, :, ct[0]:ct[0] + ct[1]].reshape(NP, -1) for ct in cts], axis=1
    )


def _lay_w1(Wq):
    """[D, H] -> [128, (jt, t, i, m)]"""
    return np.ascontiguousarray(
        Wq.reshape(TG1, 2, NP, JT_N, 128).transpose(2, 3, 0, 1, 4).reshape(NP, -1)
    )


def _lay_w2(Wq):
    """[H, D] -> [128, (jg, i, d)]"""
    return np.ascontiguousarray(
        Wq.reshape(JG_N, 2, NP, D).transpose(2, 0, 1, 3).reshape(NP, -1)
    )


def _route(xf, gate_w, gate_b):
    logits = xf @ gate_w + gate_b
    m = logits.max(-1, keepdims=True)
    ex = np.exp(logits - m)
    pb = ex / ex.sum(-1, keepdims=True)
    idx = logits.argmax(-1)
    wgt = pb[np.arange(pb.shape[0]), idx]
    return idx, wgt


_jit_cache = {}


def _run(nc, in_maps):
    """Execute nc on the 8 cores via PJRT, caching the jitted executable."""
    import jax
    from jax.sharding import Mesh, PartitionSpec
    from jax.experimental.shard_map import shard_map
    from concourse import bass2jax, mybir

    key = id(nc)
    if key not in _jit_cache:
        bass2jax.install_neuronx_cc_hook()
        pid_name = nc.partition_id_tensor.name if nc.partition_id_tensor else None
        in_names, out_names, out_avals = [], [], []
        for alloc in nc.m.functions[0].allocations:
            if not isinstance(alloc, mybir.MemoryLocationSet):
                continue
            name = alloc.memorylocations[0].name
            if alloc.kind == "ExternalInput":
                if name != pid_name:
                    in_names.append(name)
            elif alloc.kind == "ExternalOutput":
                out_names.append(name)
                out_avals.append(
                    jax.core.ShapedArray(
                        tuple(alloc.tensor_shape), mybir.dt.np(alloc.dtype)
                    )
                )
        n_params = len(in_names)
        all_names = in_names + out_names
        if pid_name is not None:
            all_names = all_names + [pid_name]

        def _body(*args):
            operands = list(args)
            if pid_name is not None:
                operands.append(bass2jax.partition_id_tensor())
            return tuple(
                bass2jax._bass_exec_p.bind(
                    *operands,
                    out_avals=tuple(out_avals),
                    in_names=tuple(all_names),
                    out_names=tuple(out_names),
                    lowering_input_output_aliases=(),
                    sim_require_finite=True,
                    sim_require_nnan=True,
                    nc=nc,
                )
            )

        mesh = Mesh(np.asarray(jax.devices()[:E]), ("core",))
        nio = n_params + len(out_names)
        sharded = jax.jit(
            shard_map(
                _body,
                mesh=mesh,
                in_specs=(PartitionSpec("core"),) * nio,
                out_specs=(PartitionSpec("core"),) * len(out_names),
                check_rep=False,
            ),
            donate_argnums=tuple(range(n_params, nio)),
            keep_unused=True,
        )
        _jit_cache[key] = (sharded, in_names, out_names, out_avals)

    sharded, in_names, out_names, out_avals = _jit_cache[key]
    concat_in = [
        np.concatenate([np.asarray(m[name]) for m in in_maps], axis=0)
        for name in in_names
    ]
    concat_zeros = [
        np.zeros((E * av.shape[0], *av.shape[1:]), av.dtype) for av in out_avals
    ]
    outs = sharded(*concat_in, *concat_zeros)
    return [
        {
            name: np.asarray(outs[i]).reshape(E, *out_avals[i].shape)[c]
            for i, name in enumerate(out_names)
        }
        for c in range(E)
    ]


def kernel(x, gate_w, gate_b, w1, b1, w2, b2):
    x = np.asarray(x, np.float32)
    gate_w = np.asarray(gate_w, np.float32)
    gate_b = np.asarray(gate_b, np.float32)
    w1 = np.asarray(w1, np.float32)
    b1 = np.asarray(b1, np.float32)
    w2 = np.asarray(w2, np.float32)
    b2 = np.asarray(b2, np.float32)

    b, s, d = x.shape
    T = b * s
    xf = x.reshape(T, d)

    idx, wgt = _route(xf, gate_w, gate_b)

    sx = _pow2_scale(np.abs(xf).max())
    sw1 = _pow2_scale(np.abs(w1).max())
    sw2 = _pow2_scale(np.abs(w2).max())
    alpha = 1.0 / (sx * sw1)

    # two-tier routing: tokens with small gate weight tolerate the 2-term
    # path (their error is scaled by wgt in the combine)
    W0 = 0.25
    tp_all = [np.nonzero((idx == e) & (wgt > W0))[0] for e in range(E)]
    tc_all = [np.nonzero((idx == e) & (wgt <= W0))[0] for e in range(E)]
    # Q floored to a multiple of 256 (no tiny cheap tail tile); overflow
    # cheap tokens take the precise path, which only improves accuracy
    QCAP = (max(len(t) for t in tc_all) // 256) * 256
    for e in range(E):
        if len(tc_all[e]) > QCAP:
            tp_all[e] = np.concatenate([tp_all[e], tc_all[e][QCAP:]])
            tc_all[e] = tc_all[e][:QCAP]
    maxc = max(len(a) + len(b) for a, b in zip(tp_all, tc_all))
    CMAX = 1280
    n_chunks = max(1, -(-maxc // CMAX))
    if n_chunks > 1:
        # fallback: all-precise chunked path
        tp_all = [np.concatenate([a, b]) for a, b in zip(tp_all, tc_all)]
        tc_all = [a[:0] for a in tp_all]

    out = np.empty((T, D), np.float32)
    for ci in range(n_chunks):
        tp = [t[ci * CMAX:(ci + 1) * CMAX] for t in tp_all]
        tc = [t[:0] if n_chunks > 1 else t for t in tc_all]
        P = max(256, -(-max(len(t) for t in tp) // 4) * 4)
        Q = -(-max(len(t) for t in tc) // 4) * 4
        C = P + Q
        cts = [(c0, n) for (c0, n) in _ctiles(P)] + [
            (P + c0, n) for (c0, n) in _ctiles(Q)]

        nc = _get_nc(P, Q, alpha)

        in_maps = []
        for e in range(E):
            xT = np.zeros((D, C), np.float32)
            np_e, nc_e = len(tp[e]), len(tc[e])
            xT[:, :np_e] = xf[tp[e]].T * sx
            xT[:, P:P + nc_e] = xf[tc[e]].T * sx
            xqh, xql = _split_fp8(xT)
            w1h, w1l = _split_fp8(w1[e] * sw1)
            w2h, w2l = _split_fp8(w2[e] * sw2)
            in_maps.append({
                "xh": _lay_x(xqh, cts),
                "xl": _lay_x(xql, cts),
                "w1h": _lay_w1(w1h),
                "w1l": _lay_w1(w1l),
                "w2h": _lay_w2(w2h),
                "w2l": _lay_w2(w2l),
                "b1t": np.ascontiguousarray(b1[e].reshape(JT_N, NP).T),
            })

        res = _run(nc, in_maps)

        for e in range(E):
            np_e, nc_e = len(tp[e]), len(tc[e])
            yT = res[e]["yT"]
            if np_e:
                y = yT[:, :np_e].T / sw2
                out[tp[e]] = wgt[tp[e], None] * (y + b2[e])
            if nc_e:
                y = yT[:, P:P + nc_e].T / sw2
                out[tc[e]] = wgt[tc[e], None] * (y + b2[e])
    return out.reshape(b, s, d)
